# revision 67
# baseline (speedup 1.0000x reference)
"""Trainium2 Bass kernel for nn_EnsembleE2EModule (moe_routing).

Reference computation (B=4096, D=784, C=10, E=1024, K=8):
  cos  = l2norm(x) @ keys.T                    [B, E]
  sims, idx = top_k(cos, 8)  (descending sims)
  gidx = sort(idx)           (ascending expert ids)
  expert_out = tanh((x @ Wm[gidx].T + bm[gidx]) / 10) * 10   [B, K, C]
  ensemble = sum_k sims_k * expert_out_k / sum_k sims_k      [B, C]
  tanh_out = tanh((x @ Wt.T + bt) / 10) * 10                 [B, C]
  vanilla  = log_softmax(x @ Wv.T + bv)                      [B, C]

Sharding: data-parallel over B across 8 NeuronCores (512 rows each);
keys / expert stack / classifier weights replicated on every core.

Since sims appear in both numerator and denominator of the ensemble,
the 1/||x|| row scaling cancels - top-k is computed on raw dot
products (same order as cosine) and the raw dot values are used as
weights directly; no normalization pass is needed.

Strategy (dense-PE): instead of gathering each selected expert's
[C, D] weight block per (sample, k) pair (64 MB of HBM traffic per
core) and reducing on DVE/ACT, compute raw[b, e, c] for ALL experts
as a dense fp16 GEMM on the Tensor engine:
    allout = [x | 1] @ [WmT | bm]     ->  [128, E*C] per sample tile
streamed over 20 PSUM column groups, with the (small) transposed
expert table streamed from DRAM group by group.  The per-tile result
round-trips through DRAM scratch so the per-sample top-8 selection
becomes 8 tiny indirect-DMA gathers of [128, C] rows (offset
p*E + expert_id), after which the weighted-tanh ensemble epilogue is
a few hundred elements per partition.

The fp32 cos / top-8 routing path is kept bit-identical to the
reference-validated formulation (raw fp32 dots on PE, DC=112 chunks):
top-8 selection near ties is sensitive to the exact arithmetic.

`dense_tiles` selects how many of the 4 sample tiles per core use the
dense-PE path; the remainder use the legacy gather path (DMA + DVE/ACT
per-pair GEMVs), trading Tensor-engine time against DMA/DVE time.
"""

import numpy as np

import concourse.bass as bass
import concourse.bacc as bacc
import concourse.tile as tile
import concourse.mybir as mybir

f32 = mybir.dt.float32
f16 = mybir.dt.float16
u32 = mybir.dt.uint32
AF = mybir.ActivationFunctionType
ALU = mybir.AluOpType
AX = mybir.AxisListType

B, D, C, E, K = 4096, 784, 10, 1024, 8
N_CORES = 8
B_SH = B // N_CORES          # 512 rows per core
P = 128                      # SBUF partitions
N_TILES = B_SH // P          # 4 sample tiles per core
DC = 112                     # contraction chunk (784 = 7 * 112)
N_CH = D // DC               # 7 chunks
R = E * C                    # 10240 dense output columns
GF = 512                     # dense group width (one PSUM bank of f32)
NG = R // GF                 # 20 dense groups
ROW = C * D                  # 7840 floats of gathered weights per expert
ROWB = ROW + C               # + C bias floats appended per expert

DENSE_TILES = 3              # tiles on the dense-PE path (rest: gather path)
N_DVE_RED = 4                # gather path: classes reduced on DVE (rest ACT)


def build_kernel(nc: bass.Bass, reps: int = 1,
                 skip_gather: bool = False, skip_compute: bool = False,
                 skip_routing: bool = False,
                 dense_tiles: int = DENSE_TILES):
    """Emit the per-core Tile program. Core-agnostic: each core gets its own
    x shard via in_maps; weights are replicated. reps>1 repeats the whole
    body (timing only: t(reps=2)-t(reps=1) cancels dispatch overhead).
    skip_* flags carve out phases for cost-model profiling only."""
    # passes over the expert table: 4 for the all-dense layout, 5 (smaller
    # resident quarters) when a legacy gather tile needs the SBUF headroom
    NQ = 4 if dense_tiles >= N_TILES else 5
    GPQ = NG // NQ
    HC = GPQ * GF
    x_d = nc.dram_tensor("x_sh", [B_SH, D], f32, kind="ExternalInput")
    xt_d = nc.dram_tensor("xt_sh", [D, B_SH], f32, kind="ExternalInput")
    keyst_d = nc.dram_tensor("keyst", [D, E], f32, kind="ExternalInput")
    # dense path: transposed expert table, one contiguous [D+1, GF] block
    # per column group; rows 0..783 = Wm[e, c, d] at column e*C + c, row
    # 784 = bm flattened (bias as an extra contraction row against the
    # ones column appended to x).
    if dense_tiles > 0:
        wtab_d = nc.dram_tensor("wtab16", [NG, D + 1, GF], f16,
                                kind="ExternalInput")
    # gather path: per-expert [C*D | C] rows
    if dense_tiles < N_TILES:
        wcat_d = nc.dram_tensor("wcat16", [E, ROWB], f16, kind="ExternalInput")
    # classifier weights, host-transposed to [DC+1, N_CH*C] fp16 with the
    # bias as contraction row DC of chunk 6 (pairs with xT16's ones row)
    wvt_d = nc.dram_tensor("wvt16", [DC + 1, N_CH * C], f16,
                           kind="ExternalInput")
    wtt_d = nc.dram_tensor("wtt16", [DC + 1, N_CH * C], f16,
                           kind="ExternalInput")

    ens_d = nc.dram_tensor("ens", [B_SH, C], f32, kind="ExternalOutput")
    tnh_d = nc.dram_tensor("tnh", [B_SH, C], f32, kind="ExternalOutput")
    van_d = nc.dram_tensor("van", [B_SH, C], f32, kind="ExternalOutput")

    with tile.TileContext(nc) as tc:
        with (
            tc.tile_pool(name="const", bufs=1) as cpool,
            tc.tile_pool(name="route", bufs=2) as rpool,
            tc.tile_pool(name="wstr", bufs=3) as wpool,
            tc.tile_pool(name="stg", bufs=3) as stpool,
            tc.tile_pool(name="gath", bufs=3) as gpool,
            tc.tile_pool(name="small", bufs=2) as spool,
            tc.tile_pool(name="ps_cos", bufs=3, space="PSUM") as ps_cos,
            tc.tile_pool(name="ps_cls", bufs=2, space="PSUM") as ps_cls,
            tc.tile_pool(name="dram", bufs=1, space="DRAM") as dpool,
        ):
          for _rep in range(reps):
            # per-partition dense gather row base: p * E (u32 -> f32)
            rowid = cpool.tile([P, 1], u32, tag="rowid")
            nc.gpsimd.iota(rowid[:], pattern=[[0, 1]], base=0,
                           channel_multiplier=E)
            rowidf = cpool.tile([P, 1], f32, tag="rowidf")
            nc.vector.tensor_copy(rowidf[:], rowid[:])



            # load order matters: the first routed tile's xT and the keysT
            # chunks gate the first cos matmuls, so they go first; the
            # remaining xT tiles / row-major copies / classifier tables
            # follow, all ahead of the (large) expert-table quarter loads.
            order = list(range(dense_tiles, N_TILES)) + list(range(dense_tiles))
            xTs, xrow = {}, {}

            def load_xt(t):
                xT = cpool.tile([DC, N_CH * P], f32, name=f"xT{t}",
                                tag=f"xT_{t}")
                nc.sync.dma_start(
                    xT[:].rearrange("p (i c) -> p i c", i=N_CH),
                    xt_d[:, t * P:(t + 1) * P].rearrange(
                        "(i p) c -> p i c", i=N_CH),
                )
                xTs[t] = xT

            load_xt(order[0])
            keysT = [
                cpool.tile([DC, E], f32, name=f"keysT{c}", tag=f"keysT{c}")
                for c in range(N_CH)
            ]
            for c in range(N_CH):
                nc.sync.dma_start(keysT[c][:], keyst_d[c * DC:(c + 1) * DC, :])
            for t in order[1:]:
                load_xt(t)
            for t in range(dense_tiles, N_TILES):
                x_t = cpool.tile([P, D], f32, name=f"x{t}", tag=f"x_{t}")
                nc.sync.dma_start(x_t[:], x_d[t * P:(t + 1) * P, :])
                xrow[t] = x_t
            wvT = cpool.tile([DC + 1, N_CH * C], f16, tag="wvT")
            nc.sync.dma_start(wvT[:], wvt_d[:])
            wtT = cpool.tile([DC + 1, N_CH * C], f16, tag="wtT")
            nc.sync.dma_start(wtT[:], wtt_d[:])

            # ---- phase 1: routing (+classifiers) per sample tile;
            # legacy gather-path tiles are routed first so their (large)
            # expert gathers run during the remaining routing work, when
            # the DMA engines are otherwise idle ----
            xT16s, offss, w_ts, ascs, x16s = {}, {}, {}, {}, {}
            for t in order:
                dense = t < dense_tiles
                xT = xTs[t]

                # fp16 copy of xT with a ones row at partition DC of
                # chunk 6 (pairs with the bias rows of the fp16 weight
                # tables).  memset everything to 1.0 first: the copies
                # overwrite partitions 0..DC-1 and engine ops cannot
                # start at partition 112 for a targeted memset.
                xT16 = cpool.tile([DC + 1, N_CH * P], f16, tag=f"xT16_{t}")
                nc.vector.memset(xT16[:, :], 1.0)
                nc.vector.tensor_copy(xT16[:DC, :], xT[:])

                # cos = x @ keys.T  (raw dots; row scaling cancels)
                cos_t = rpool.tile([P, E], f32, tag="cos")
                if skip_routing:
                    nc.vector.memset(cos_t[:], 0.0)
                for h in range(2) if not skip_routing else []:
                    pc = ps_cos.tile([P, E // 2], f32, tag="pcos")
                    for c in range(N_CH):
                        nc.tensor.matmul(
                            pc[:],
                            lhsT=xT[:, c * P:(c + 1) * P],
                            rhs=keysT[c][:, h * (E // 2):(h + 1) * (E // 2)],
                            start=(c == 0),
                            stop=(c == N_CH - 1),
                        )
                    nc.vector.tensor_copy(cos_t[:, h * (E // 2):(h + 1) * (E // 2)], pc[:])

                # top-8 (descending) + indices
                w_t = cpool.tile([P, K], f32, tag=f"w_{t}")
                idx_t = rpool.tile([P, K], u32, tag="idx")
                nc.vector.max(out=w_t[:], in_=cos_t[:])
                nc.vector.max_index(out=idx_t[:], in_max=w_t[:], in_values=cos_t[:])

                # ascending expert ids: u32 -> f32, max8 (desc), reverse
                idxf = rpool.tile([P, K], f32, tag="idxf")
                nc.vector.tensor_copy(idxf[:], idx_t[:])
                dsc = rpool.tile([P, K], f32, tag="dsc")
                nc.vector.max(out=dsc[:], in_=idxf[:])
                if dense:
                    # dense gather offsets: p*E + ascending expert id
                    offsf = rpool.tile([P, K], f32, tag="offsf")
                    nc.vector.tensor_scalar(
                        out=offsf[:], in0=dsc[:, ::-1],
                        scalar1=rowidf[:, 0:1], scalar2=None, op0=ALU.add,
                    )
                    offs = cpool.tile([P, K], u32, tag=f"offs_{t}")
                    nc.vector.tensor_copy(offs[:], offsf[:])
                    offss[t] = offs
                else:
                    asc = cpool.tile([P, K], u32, tag=f"asc_{t}")
                    nc.vector.tensor_copy(asc[:], dsc[:, ::-1])
                    ascs[t] = asc
                    x16 = cpool.tile([P, D], f16, tag=f"x16_{t}")
                    nc.vector.tensor_copy(x16[:], xrow[t][:])
                    x16s[t] = x16
                xT16s[t] = xT16
                w_ts[t] = w_t


            # ---- classifiers (fp16, bias folded as contraction row) ----
            def emit_classifiers(t):
                xT16 = xT16s[t]
                for which, (wT, out_d) in enumerate(
                    ((wvT, van_d), (wtT, tnh_d))
                ):
                    pl = ps_cls.tile([P, C], f32, tag="pcls")
                    for c in range(N_CH):
                        rows = DC + 1 if c == N_CH - 1 else DC
                        nc.tensor.matmul(
                            pl[:],
                            lhsT=xT16[:rows, c * P:(c + 1) * P],
                            rhs=wT[:rows, c * C:(c + 1) * C],
                            start=(c == 0),
                            stop=(c == N_CH - 1),
                        )
                    logits = spool.tile([P, C], f32, name="logits",
                                        tag=f"log{which}")
                    nc.vector.tensor_copy(logits[:], pl[:])
                    if which == 1:
                        # tanh_out = tanh(logits/10)*10
                        th = spool.tile([P, C], f32, name="th", tag="th")
                        nc.scalar.activation(th[:], logits[:], AF.Tanh,
                                             scale=0.1)
                        out_t = spool.tile([P, C], f32, name="out_t",
                                           tag="tout")
                        nc.vector.tensor_scalar_mul(out_t[:], th[:], 10.0)
                    else:
                        # vanilla = log_softmax(logits)
                        mx = spool.tile([P, 1], f32, name="mx", tag="mx")
                        nc.vector.tensor_reduce(
                            out=mx[:], in_=logits[:], axis=AX.X, op=ALU.max
                        )
                        sh = spool.tile([P, C], f32, name="sh", tag="sh")
                        nc.vector.tensor_scalar(
                            out=sh[:], in0=logits[:], scalar1=mx[:, 0:1],
                            scalar2=None, op0=ALU.subtract,
                        )
                        ex = spool.tile([P, C], f32, name="ex", tag="ex")
                        se = spool.tile([P, 1], f32, name="se", tag="se")
                        nc.scalar.activation(ex[:], sh[:], AF.Exp,
                                             accum_out=se[:])
                        lse = spool.tile([P, 1], f32, name="lse", tag="lse")
                        nc.scalar.activation(lse[:], se[:], AF.Ln)
                        out_t = spool.tile([P, C], f32, name="out_t",
                                           tag="vout")
                        nc.vector.tensor_scalar(
                            out=out_t[:], in0=sh[:], scalar1=lse[:, 0:1],
                            scalar2=None, op0=ALU.subtract,
                        )
                    nc.sync.dma_start(out_d[t * P:(t + 1) * P, :], out_t[:])

            # ---- shared ensemble tail ----
            def emit_ensemble(t, w_t, ens_num):
                # ensemble = 10 * sum_k w*tanh / sum_k w  (the 10x from
                # tanh's scale is folded into wsum's 0.1 factor)
                wsum = spool.tile([P, 1], f32, name="wsum", tag="wsum")
                nc.vector.tensor_reduce(out=wsum[:], in_=w_t[:], axis=AX.X,
                                        op=ALU.add)
                nc.vector.tensor_scalar_mul(wsum[:], wsum[:], 0.1)
                winv = spool.tile([P, 1], f32, name="winv", tag="winv")
                nc.vector.reciprocal(winv[:], wsum[:])
                ens_t = spool.tile([P, C], f32, name="ens_t", tag="ens")
                nc.vector.tensor_scalar_mul(ens_t[:], ens_num[:], winv[:, 0:1])
                nc.sync.dma_start(ens_d[t * P:(t + 1) * P, :], ens_t[:])

            # ---- phase 2: dense expert GEMM in four tile-major passes
            # over resident expert-table quarters (double-buffered), so
            # each tile's scratch halves complete progressively and the
            # selection gathers + epilogues overlap later passes instead
            # of serializing behind the final writes ----
            if dense_tiles > 0:
                allouts = [
                    dpool.tile([P * E, C], f16, name=f"allout{t}",
                               tag=f"allout{t}")
                    for t in range(dense_tiles)
                ]
                aviews = [
                    a[:].rearrange("(b e) c -> b (e c)", b=P) for a in allouts
                ]

                def load_quarter(q):
                    # per source group: rows 0..783 as 7 chunk column
                    # blocks in one 3-dim strided DMA; bias row 784 lands
                    # at partition DC of chunk 6.
                    tq = wpool.tile([DC + 1, N_CH * HC], f16, name="tq",
                                    tag="tq")
                    tq4 = tq[:DC, :].rearrange("p (i g c) -> p i g c",
                                               i=N_CH, g=GPQ)
                    for g2 in range(GPQ):
                        # two DMAs per group: shorter exclusive holds on
                        # the DMA engines interleave better with the
                        # stage writes and expert gathers
                        src = wtab_d[q * GPQ + g2, :D, :].rearrange(
                            "(i p) c -> p i c", i=N_CH)
                        for i0, i1 in ((0, 2), (2, 4), (4, 6), (6, 7)):
                            nc.sync.dma_start(tq4[:, i0:i1, g2, :],
                                              src[:, i0:i1, :])
                    nc.sync.dma_start(
                        tq[DC:DC + 1, (N_CH - 1) * HC:].rearrange(
                            "p (g c) -> p g c", g=GPQ),
                        wtab_d[q * GPQ:(q + 1) * GPQ, D:D + 1, :].rearrange(
                            "g p c -> p g c"),
                    )
                    return tq

                tq_next = load_quarter(0)
                wgs = [None] * dense_tiles
                for q in range(NQ):
                    tq = tq_next
                    if q + 1 < NQ:
                        tq_next = load_quarter(q + 1)
                    if q < dense_tiles:
                        emit_classifiers(q)
                    col0 = q * HC
                    for t in range(dense_tiles):
                        stage = stpool.tile([P, HC], f16, name="stage",
                                            tag="stq",
                                            bufs=3 if dense_tiles >= N_TILES
                                            else 2)
                        if skip_compute:
                            nc.vector.memset(stage[:], 0.0)
                        else:
                            xT16 = xT16s[t]
                            for g2 in range(GPQ):
                                pd = ps_cos.tile([P, GF], f32, tag="pcos")
                                for c in range(N_CH):
                                    rows = DC + 1 if c == N_CH - 1 else DC
                                    nc.tensor.matmul(
                                        pd[:],
                                        lhsT=xT16[:rows, c * P:(c + 1) * P],
                                        rhs=tq[:rows, c * HC + g2 * GF:
                                               c * HC + (g2 + 1) * GF],
                                        start=(c == 0),
                                        stop=(c == N_CH - 1),
                                    )
                                nc.scalar.copy(
                                    stage[:, g2 * GF:(g2 + 1) * GF], pd[:])
                        if q == NQ - 1:
                            for g2 in range(GPQ):
                                nc.sync.dma_start(
                                    aviews[t][:, col0 + g2 * GF:
                                              col0 + (g2 + 1) * GF],
                                    stage[:, g2 * GF:(g2 + 1) * GF])
                        else:
                            h2 = HC // 2
                            nc.sync.dma_start(
                                aviews[t][:, col0:col0 + h2], stage[:, :h2])
                            nc.sync.dma_start(
                                aviews[t][:, col0 + h2:col0 + HC],
                                stage[:, h2:])
                        if q == NQ - 1:
                            # tile t's scratch is complete: K selection
                            # gathers, then its ensemble epilogue — both
                            # overlap the remaining tiles' matmul passes.
                            wgs[t] = cpool.tile([P, K * C], f16,
                                                name=f"wg{t}", tag=f"wg_{t}")
                            if skip_gather:
                                nc.vector.memset(wgs[t][:], 0.0)
                            else:
                                for k in range(K):
                                    nc.gpsimd.indirect_dma_start(
                                        out=wgs[t][:, k * C:(k + 1) * C],
                                        out_offset=None,
                                        in_=allouts[t][:],
                                        in_offset=bass.IndirectOffsetOnAxis(
                                            ap=offss[t][:, k:k + 1],
                                            axis=0),
                                    )
                            # ---- dense ensemble epilogue for tile t ----
                            wg = wgs[t]
                            eo = spool.tile([P, K * C], f32, name="eo",
                                            tag="eo")
                            nc.scalar.activation(eo[:], wg[:], AF.Tanh,
                                                 scale=0.1)
                            prod = spool.tile([P, K * C], f32, name="prod",
                                              tag="prod")
                            nc.vector.tensor_tensor(
                                out=prod[:].rearrange("p (k c) -> p k c", k=K),
                                in0=w_ts[t][:].unsqueeze(2).to_broadcast(
                                    [P, K, C]),
                                in1=eo[:].rearrange("p (k c) -> p k c", k=K),
                                op=ALU.mult,
                            )
                            ens_num = spool.tile([P, C], f32, name="ens_num",
                                                 tag="ensn")
                            nc.vector.tensor_reduce(
                                out=ens_num[:],
                                in_=prod[:].rearrange("p (k c) -> p c k", k=K),
                                axis=AX.X,
                                op=ALU.add,
                            )
                            emit_ensemble(t, w_ts[t], ens_num)

            # ---- phase 3: legacy gather-path tiles ----
            for t in range(dense_tiles, N_TILES):
                emit_classifiers(t)
                w_t = w_ts[t]
                if True:
                    # legacy gather path: fetch each selected expert's
                    # [C*D | C] row and reduce the per-pair GEMVs on
                    # DVE (products + N_DVE_RED class reductions) and
                    # ACT (copy-accumulate for the remaining classes).
                    asc, x16 = ascs[t], x16s[t]
                    raw = spool.tile([P, C * K], f32, tag="raw")
                    biasg = spool.tile([P, K * C], f32, tag="biasg")
                    rawD = spool.tile([P, K * max(N_DVE_RED, 1)], f16, tag="rawD")
                    for k in range(K):
                        wgk = gpool.tile([P, ROWB], f16, tag="wgk", bufs=2)
                        if skip_gather:
                            nc.vector.memset(wgk[:, ROW:], 0.0)
                        else:
                            nc.gpsimd.indirect_dma_start(
                                out=wgk[:],
                                out_offset=None,
                                in_=wcat_d[:],
                                in_offset=bass.IndirectOffsetOnAxis(
                                    ap=asc[:, k:k + 1], axis=0),
                            )
                        prodk = wgk  # product computed in place over the gather
                        nc.vector.tensor_tensor(
                            out=prodk[:, :ROW].rearrange("p (c d) -> p c d", c=C),
                            in0=x16[:].unsqueeze(1).to_broadcast([P, C, D]),
                            in1=wgk[:, :ROW].rearrange("p (c d) -> p c d", c=C),
                            op=ALU.mult,
                        )
                        for c in range(N_DVE_RED, C):
                            nc.scalar.activation(
                                out=prodk[:, c * D:(c + 1) * D],
                                in_=prodk[:, c * D:(c + 1) * D],
                                func=AF.Copy,
                                accum_out=raw[:, c * K + k:c * K + k + 1],
                            )
                        if N_DVE_RED:
                            with nc.allow_low_precision(
                                    reason="fp16 partial sums match the "
                                    "fp16 gather/product precision"):
                                nc.vector.tensor_reduce(
                                    out=rawD[:, k * N_DVE_RED:
                                             (k + 1) * N_DVE_RED],
                                    in_=prodk[:, :N_DVE_RED * D].rearrange(
                                        "p (c d) -> p c d", c=N_DVE_RED),
                                    axis=AX.X,
                                    op=ALU.add,
                                )
                        nc.vector.tensor_copy(
                            biasg[:, k * C:(k + 1) * C], wgk[:, ROW:ROW + C]
                        )
                    if N_DVE_RED:
                        nc.vector.tensor_copy(
                            raw[:, :N_DVE_RED * K].rearrange(
                                "p (c k) -> p c k", c=N_DVE_RED),
                            rawD[:].rearrange("p (k c) -> p c k", k=K),
                        )
                    nc.vector.tensor_add(
                        raw[:].rearrange("p (c k) -> p c k", c=C),
                        raw[:].rearrange("p (c k) -> p c k", c=C),
                        biasg[:].rearrange("p (k c) -> p c k", k=K),
                    )
                    eo = spool.tile([P, C * K], f32, tag="eo")
                    nc.scalar.activation(eo[:], raw[:], AF.Tanh, scale=0.1)
                    prod = spool.tile([P, C * K], f32, tag="prod")
                    nc.vector.tensor_tensor(
                        out=prod[:].rearrange("p (c k) -> p c k", c=C),
                        in0=w_t[:].unsqueeze(1).to_broadcast([P, C, K]),
                        in1=eo[:].rearrange("p (c k) -> p c k", c=C),
                        op=ALU.mult,
                    )
                    ens_num = spool.tile([P, C], f32, tag="ensn")
                    nc.vector.tensor_reduce(
                        out=ens_num[:],
                        in_=prod[:].rearrange("p (c k) -> p c k", c=C),
                        axis=AX.X,
                        op=ALU.add,
                    )
                emit_ensemble(t, w_t, ens_num)

    nc.finalize()
    return nc


def make_in_maps(x, keys, Wm, bm, Wv, bv, Wt, bt):
    """Host-side marshalling only: shard x over cores, replicate weights,
    lay out the expert stack for the device (pure layout + fp16 cast)."""
    common = dict(
        keyst=np.ascontiguousarray(np.asarray(keys, np.float32).T),
    )
    if DENSE_TILES > 0:
        # [D, E*C] with column e*C + c = Wm[e, c, d]; bias row appended;
        # then regrouped to [NG, D+1, GF] so each group is contiguous.
        wt_flat = np.ascontiguousarray(
            np.asarray(Wm, np.float32).transpose(2, 0, 1).reshape(D, R))
        tab = np.concatenate(
            [wt_flat, np.asarray(bm, np.float32).reshape(1, R)], axis=0)
        tab16 = tab.astype(np.float16)                      # [D+1, R]
        common["wtab16"] = np.ascontiguousarray(
            tab16.reshape(D + 1, NG, GF).transpose(1, 0, 2))
    if DENSE_TILES < N_TILES:
        wcat = np.concatenate(
            [np.ascontiguousarray(Wm, np.float32).reshape(E, ROW),
             np.ascontiguousarray(bm, np.float32)], axis=1)
        common["wcat16"] = wcat.astype(np.float16)
    def cls16(W, b):
        # [DC+1, N_CH*C] fp16: chunk-major transposed weights with the
        # bias as contraction row DC of chunk 6 (pairs with xT16's ones)
        T = np.ones((DC + 1, N_CH * C), np.float32)
        WT = np.ascontiguousarray(W, np.float32).T          # [D, C]
        for i in range(N_CH):
            T[:DC, i * C:(i + 1) * C] = WT[i * DC:(i + 1) * DC]
        T[DC, (N_CH - 1) * C:] = np.asarray(b, np.float32).reshape(C)
        return T.astype(np.float16)

    common.update(
        wvt16=cls16(Wv, bv),
        wtt16=cls16(Wt, bt),
    )
    x = np.ascontiguousarray(x, np.float32)
    return [
        dict(x_sh=x[c * B_SH:(c + 1) * B_SH],
             xt_sh=np.ascontiguousarray(x[c * B_SH:(c + 1) * B_SH].T),
             **common) for c in range(N_CORES)
    ]


_CACHED = {}


def _get_nc(reps: int = 1):
    key = f"nc{reps}"
    if key not in _CACHED:
        nc = bacc.Bacc(debug=False)
        build_kernel(nc, reps=reps)
        _CACHED[key] = nc
    return _CACHED[key]


def kernel(x, keys, Wm, bm, Wv, bv, Wt, bt):
    from concourse.bass_utils import run_bass_kernel_spmd

    nc = _get_nc()
    in_maps = make_in_maps(x, keys, Wm, bm, Wv, bv, Wt, bt)
    res = None
    for attempt in range(3):
        try:
            res = run_bass_kernel_spmd(
                nc, in_maps, core_ids=list(range(N_CORES))).results
            break
        except Exception:
            # transient device/runtime hiccups recover on re-execution
            if attempt == 2:
                raise
    assert res is not None
    ensemble = np.concatenate([res[c]["ens"] for c in range(N_CORES)], axis=0)
    tanh_out = np.concatenate([res[c]["tnh"] for c in range(N_CORES)], axis=0)
    vanilla = np.concatenate([res[c]["van"] for c in range(N_CORES)], axis=0)
    return ensemble, tanh_out, vanilla
